# revision 10
# baseline (speedup 1.0000x reference)
"""Trainium2 Bass kernel for nn_Attention_42279658062045 (gnn_message_passing).

Computes, for each of B=200000 nodes:
    simi   = exp(-source_distance^2 / 2)                  [B, K]
    weight = softmax(simi @ kernel + bias, axis=-1)       [B, K]
    mean   = einsum('bk,bkd->bd', weight, context)        [B, D]

Sharding: pure data parallel over the node axis B across 8 NeuronCores;
kernel/bias replicated; no cross-device communication.

Per-core dataflow (B_LOCAL = 25000 rows, tiles of 128 rows, f32 throughout):
  - HWDGE streams context in 2-tile (~2 MB) chunks (dominant HBM traffic).
  - PE: transpose of simi tiles; logits = simi @ kernel + bias via two
    accumulating matmuls (ones-row stationary adds the bias).
  - ACT: batched square+exp of all distances, PSUM->SBUF copy of simi^T,
    exp(logits) with accum_out giving the softmax denominator, and the
    weighted product for 5 of the 30 k-slabs (per-partition scale).
  - DVE: reciprocal, weight normalize, weighted product for 25 k-slabs,
    and the final k-reduction.
  - GPSIMD: one batched fold (k -> k/2) per 4-tile chunk.

The product tensor uses an interleaved layout [d_hi(32), k(30), d_lo(2)]
(flat addr = d_hi*60 + k*2 + d_lo) so the k-reduction reads at 8-byte
stride, which the DVE streams at full rate (256-byte strides cost ~1.6x).
"""

import numpy as np

N_CORES = 8
B, K, D = 200000, 30, 64
B_LOCAL = B // N_CORES  # 25000
P = 128
CT = 4          # tiles per context DMA chunk
PT = 4          # tiles per product chunk (one GPSIMD fold per chunk)
IL = 2          # product interleave: [d_hi(32), k(30), d_lo(IL)]
DH = D // IL    # 32
KH = K // 2     # 15 (fold halves)
ACT_SLABS = 3   # k-slabs whose product is computed on the scalar engine
DVE_SLABS = K - ACT_SLABS
KR = 8          # slabs left for the DVE reduce after two GPSIMD folds

_CACHE = {}


def _build():
    import concourse.bacc as bacc
    import concourse.tile as tile
    from concourse import mybir
    from concourse.masks import make_identity

    fp32 = mybir.dt.float32
    AF = mybir.ActivationFunctionType

    nc = bacc.Bacc("TRN2", target_bir_lowering=False, debug=False,
                   num_devices=N_CORES)

    dist = nc.dram_tensor("source_distance", [B_LOCAL, K], fp32,
                          kind="ExternalInput").ap()
    ctx_d = nc.dram_tensor("context", [B_LOCAL, K, D], fp32,
                           kind="ExternalInput").ap()
    kern = nc.dram_tensor("kernel", [K, K], fp32, kind="ExternalInput").ap()
    bias = nc.dram_tensor("bias", [K], fp32, kind="ExternalInput").ap()
    out = nc.dram_tensor("out", [B_LOCAL, D], fp32, kind="ExternalOutput").ap()

    n_full = B_LOCAL // P          # 195 full tiles
    rem = B_LOCAL - n_full * P     # 40 leftover rows

    dist_v = dist[:n_full * P, :].rearrange("(n p) k -> p n k", p=P)
    ctx_v = ctx_d[:n_full * P].rearrange("(n p) k d -> p n (k d)", p=P)
    out_v = out[:n_full * P, :].rearrange("(n p) d -> p n d", p=P)

    with tile.TileContext(nc) as tc:
        from contextlib import ExitStack
        with ExitStack() as st:
            consts = st.enter_context(tc.tile_pool(name="consts", bufs=1))
            big = st.enter_context(tc.tile_pool(name="big", bufs=1))
            ctxp = st.enter_context(tc.tile_pool(name="ctx", bufs=2))
            prodp = st.enter_context(tc.tile_pool(name="prod", bufs=2))
            small = st.enter_context(tc.tile_pool(name="small", bufs=3))
            rzp = st.enter_context(tc.tile_pool(name="rz", bufs=10))
            psum_t = st.enter_context(
                tc.tile_pool(name="psumT", bufs=2, space="PSUM"))
            psum_l = st.enter_context(
                tc.tile_pool(name="psumL", bufs=2, space="PSUM"))

            ident = consts.tile([P, P], fp32)
            make_identity(nc, ident)
            kern_s = consts.tile([K, K], fp32)
            nc.sync.dma_start(out=kern_s, in_=kern)
            bias_s = consts.tile([1, K], fp32)
            nc.sync.dma_start(out=bias_s, in_=bias.unsqueeze(0))
            ones_s = consts.tile([1, P], fp32)
            nc.vector.memset(ones_s, 1.0)

            # All distances for the full tiles; squared+exp'd in place.
            simi_all = big.tile([P, n_full, K], fp32)
            nc.sync.dma_start(out=simi_all, in_=dist_v)
            nc.scalar.activation(out=simi_all, in_=simi_all, func=AF.Square)
            nc.scalar.activation(out=simi_all, in_=simi_all, func=AF.Exp,
                                 scale=-0.5)

            # Staged output for the full tiles (one big DMA at the end).
            mean_all = big.tile([P, n_full, D], fp32)

            def softmax_exp(simi_ap, expw_ap, rows):
                """simi [rows, K] -> expw (unnormalized softmax numerator)
                written to expw_ap, returns rz = 1/sum(expw) [rows, 1].

                No DVE op here reads a per-partition scalar operand
                (TensorScalarPtr): those hard-block for the whole duration
                of any concurrent GPSIMD op (shared SBUF port, measured
                6.8us stalls), so normalization is applied on the scalar
                engine at the very end instead.
                """
                simiT_p = psum_t.tile([K, P], fp32, tag="simiT_p")
                nc.tensor.transpose(out=simiT_p[:, :rows], in_=simi_ap,
                                    identity=ident[:rows, :rows])
                simiT_s = small.tile([K, P], fp32, tag="simiT_s")
                nc.scalar.copy(out=simiT_s[:, :rows], in_=simiT_p[:, :rows])

                logits_p = psum_l.tile([P, K], fp32, tag="logits_p")
                nc.tensor.matmul(out=logits_p[:rows, :],
                                 lhsT=simiT_s[:, :rows], rhs=kern_s,
                                 start=True, stop=False)
                nc.tensor.matmul(out=logits_p[:rows, :],
                                 lhsT=ones_s[:, :rows], rhs=bias_s,
                                 start=False, stop=True)

                zsum = small.tile([P, 1], fp32, tag="zsum")
                nc.scalar.activation(out=expw_ap, in_=logits_p[:rows, :],
                                     func=AF.Exp, accum_out=zsum[:rows, :])
                rz = rzp.tile([P, 1], fp32, tag="rz")
                nc.vector.reciprocal(out=rz[:rows, :], in_=zsum[:rows, :])
                return rz

            # Full tiles: context DMA per PT-tile chunk; products in PT-tile
            # interleaved chunks; one GPSIMD product + one fold per chunk.
            for c0 in range(0, n_full, PT):
                pn = min(PT, n_full - c0)
                prod = prodp.tile([P, PT, DH, K, IL], fp32, tag="prod")
                ew = prodp.tile([P, PT, K], fp32, tag="ew")
                rzs = []
                ctx_tile = ctxp.tile([P, CT, K * D], fp32, tag="ctx")
                nc.sync.dma_start(out=ctx_tile[:, :pn, :],
                                  in_=ctx_v[:, c0:c0 + pn, :])
                for j in range(pn):
                    t = c0 + j
                    ctx3 = ctx_tile[:, j, :].rearrange("p (k d) -> p k d", k=K)

                    rz = softmax_exp(simi_all[:, t, :], ew[:, j, :], P)
                    rzs.append(rz)

                    # DVE: product for slabs [0, DVE_SLABS), interleaved out
                    nc.vector.tensor_mul(
                        out=prod[:, j].rearrange("p h k l -> p k h l")[:, :DVE_SLABS],
                        in0=ctx3[:, :DVE_SLABS, :],
                        in1=ew[:, j, :DVE_SLABS].unsqueeze(2).broadcast_to(
                            [P, DVE_SLABS, D]))
                    # ACT: product for the last ACT_SLABS slabs
                    for k in range(DVE_SLABS, K):
                        nc.scalar.mul(out=prod[:, j, :, k, :],
                                      in_=ctx3[:, k, :], mul=ew[:, j, k:k + 1])

                # GPSIMD fold 1: k -> k+15 pairs (30 slabs -> 15)
                nc.gpsimd.tensor_add(out=prod[:, :pn, :, 0:KH, :],
                                     in0=prod[:, :pn, :, 0:KH, :],
                                     in1=prod[:, :pn, :, KH:K, :])
                # GPSIMD fold 2: k -> k+8 pairs (slabs 0..6 += 8..14),
                # leaving slabs 0..7 for the DVE reduce.
                nc.gpsimd.tensor_add(out=prod[:, :pn, :, 0:KR - 1, :],
                                     in0=prod[:, :pn, :, 0:KR - 1, :],
                                     in1=prod[:, :pn, :, KR:KH, :])

                # DVE: reduce the remaining KR slabs per tile; ACT: normalize
                for j in range(pn):
                    t = c0 + j
                    nc.vector.reduce_sum(
                        out=mean_all[:, t, :].rearrange("p (h l) -> p h l", l=IL),
                        in_=prod[:, j].rearrange("p h k l -> p h l k")[:, :, :, 0:KR],
                        axis=mybir.AxisListType.X)
                    nc.scalar.mul(out=mean_all[:, t, :], in_=mean_all[:, t, :],
                                  mul=rzs[j])

            nc.sync.dma_start(out=out_v, in_=mean_all)

            # Remainder rows (partial tile): simple all-DVE path.
            if rem:
                simi_r = small.tile([P, K], fp32, tag="simi_r")
                nc.sync.dma_start(out=simi_r[:rem, :], in_=dist[n_full * P:, :])
                nc.scalar.activation(out=simi_r[:rem, :], in_=simi_r[:rem, :],
                                     func=AF.Square)
                nc.scalar.activation(out=simi_r[:rem, :], in_=simi_r[:rem, :],
                                     func=AF.Exp, scale=-0.5)
                ctx_r = ctxp.tile([P, K * D], fp32, tag="ctx")
                nc.sync.dma_start(
                    out=ctx_r[:rem, :],
                    in_=ctx_d[n_full * P:].rearrange("b k d -> b (k d)"))
                expw_r = small.tile([P, K], fp32, tag="expw_r")
                rz_r = softmax_exp(simi_r[:rem, :], expw_r[:rem, :], rem)
                prod_r = prodp.tile([P, K, D], fp32, tag="prod")
                ctx3r = ctx_r[:rem, :].rearrange("p (k d) -> p k d", k=K)
                nc.vector.tensor_mul(
                    out=prod_r[:rem], in0=ctx3r,
                    in1=expw_r[:rem, :].unsqueeze(2).broadcast_to([rem, K, D]))
                mean_r = small.tile([P, D], fp32, tag="mean_r")
                nc.vector.reduce_sum(
                    out=mean_r[:rem, :],
                    in_=prod_r[:rem].rearrange("p k d -> p d k"),
                    axis=mybir.AxisListType.X)
                nc.scalar.mul(out=mean_r[:rem, :], in_=mean_r[:rem, :],
                              mul=rz_r[:rem, :])
                nc.sync.dma_start(out=out[n_full * P:, :], in_=mean_r[:rem, :])

    nc.compile()
    return nc


def _get_nc():
    if "nc" not in _CACHE:
        _CACHE["nc"] = _build()
    return _CACHE["nc"]


def kernel(source_distance, context, kernel, bias, _trace=False, _tmpdir=None):
    from concourse.bass_utils import run_bass_kernel_spmd

    nc = _get_nc()

    source_distance = np.ascontiguousarray(source_distance, dtype=np.float32)
    context = np.ascontiguousarray(context, dtype=np.float32)
    kernel = np.ascontiguousarray(kernel, dtype=np.float32)
    bias = np.ascontiguousarray(bias, dtype=np.float32)

    in_maps = []
    for i in range(N_CORES):
        lo, hi = i * B_LOCAL, (i + 1) * B_LOCAL
        in_maps.append({
            "source_distance": source_distance[lo:hi],
            "context": context[lo:hi],
            "kernel": kernel,
            "bias": bias,
        })

    res = run_bass_kernel_spmd(nc, in_maps, list(range(N_CORES)),
                               trace=_trace, tmpdir=_tmpdir)
    out = np.concatenate([res.results[i]["out"] for i in range(N_CORES)], axis=0)
    if _trace:
        _CACHE["last_results"] = res
    return out
